# revision 15
# baseline (speedup 1.0000x reference)
"""Multi-head attention (B=2, N=2048, E=1024, H=16) on 8 trn2 NeuronCores.

Sharding: data-parallel over batch (2 groups of 4 cores) x tensor-parallel
over heads (4 heads per core). Each core computes, for its batch b and its
4 heads: qkv projection (its W_qkv column slices), head-parallel attention,
and a partial output projection (its W_out row slice), returning a partial
y^T [1024, 2048]. The host sums the 4 partials per batch and transposes.

Device layout (no transposes anywhere on device):
  - x is passed pre-transposed as xT [E, N] (host transpose).
  - q, k are produced d-major in bf16: qkT [128, 4, N]: partition = d of a
    head PAIR (rows 0:64 head-lo, 64:128 head-hi); groups = [q pair0,
    q pair1, k pair0, k pair1]. The qkv projection itself runs in f32r
    (full f32 inputs, relaxed single-pass matmul) so only the final
    rounding of q/k/v is bf16.
  - v is produced tok-major: V [128(tok), 16(tok-tile), 256(head d)] bf16.
  - scores are computed transposed S^T[j, i] = k_j . q_i via bf16 matmuls,
    two heads concurrently via K=64 row groups.
  - softmax: exp on ScalarE (scale folded in, no max subtraction -- scores
    are O(1) by construction); denominator via ones-column matmuls (M=1,
    col-groups 0/32/64/96); P^T is bf16.
  - attention out Z is staged unnormalized to SBUF; all reciprocals are
    batched at the end on a [128, 64] repack (DVE reciprocal is ~8
    cycles/elem/lane, so use all 128 lanes), then broadcast via a DRAM
    bounce and applied as a multiply.
  - out-projection computes y^T [e, i] (f32r) with W_out stationary, bias
    added per-partition during the PSUM->SBUF copy.

Emission is software-pipelined: within the attention loop, round r's
S-matmuls are emitted before round r-1's AV/den matmuls so the PE never
head-of-line blocks on the exp that feeds AV.
"""

import numpy as np
import ml_dtypes

import concourse.mybir as mybir
import concourse.tile as tile
from concourse import bacc
from concourse.bass_utils import run_bass_kernel_spmd

F32R = mybir.dt.float32r
F32 = mybir.dt.float32
BF16 = mybir.dt.bfloat16

E = 1024  # embed dim
N = 2048  # sequence length
DH = 64  # head dim
SCALE = DH**-0.5
IC = 512  # i-chunk (queries per block)
JT = N // 128  # 16 j-tiles (keys)
JG = 2  # j-tiles per exp group (ACT free dim = JG*IC = 1024)
NCORES = 8

_NC = None  # cached compiled program (same for every core / call)


def _build():
    nc = bacc.Bacc("TRN2", target_bir_lowering=False, debug=False)
    xT_d = nc.dram_tensor("xT", [E, N], F32R, kind="ExternalInput")
    wqk_d = nc.dram_tensor("Wqk", [E, 512], F32R, kind="ExternalInput")
    wv_d = nc.dram_tensor("Wv", [E, 256], F32R, kind="ExternalInput")
    bqk_d = nc.dram_tensor("bqk", [128, 4], F32, kind="ExternalInput")
    bv_d = nc.dram_tensor("bv", [1, 256], F32, kind="ExternalInput")
    wout_d = nc.dram_tensor("Wout", [256, E], F32R, kind="ExternalInput")
    bout_d = nc.dram_tensor("bout", [128, 8], F32, kind="ExternalInput")
    ones_d = nc.dram_tensor("ones", [128, 1], BF16, kind="ExternalInput")
    yT_d = nc.dram_tensor("yT", [E, N], F32, kind="ExternalOutput")
    scr_d = nc.dram_tensor("den_scratch", [4, N], F32)  # internal bounce
    scr2_d = nc.dram_tensor("recip_scratch", [4, N], F32)  # internal bounce

    with tile.TileContext(nc) as tc:
        with (
            tc.tile_pool(name="persist", bufs=1) as persist,
            tc.tile_pool(name="xpool", bufs=1) as xpool,
        ):
            # ---- persistent SBUF tensors ----
            wqk = persist.tile([128, 8, 512], F32R)
            wv = persist.tile([128, 8, 256], F32R)
            bqk = persist.tile([128, 4], F32)
            bvB = persist.tile([128, 256], F32)
            wout = persist.tile([128, 2, E], F32R)
            bout = persist.tile([128, 8], F32)
            ones = persist.tile([128, 1], BF16)
            qkT = persist.tile([128, 4, N], BF16)  # q/k d-major
            Vsb = persist.tile([128, JT, 256], BF16)  # v tok-major
            Zsb = persist.tile([128, 2, N], F32R)  # unnormalized attn out
            O2T = persist.tile([128, 2, N], F32R)  # normalized attn out
            rp = persist.tile([128, DH], F32)  # packed denominators
            densb = persist.tile([97, IC], F32)  # den staging (rows 0/32/64/96)
            rq = persist.tile([128, DH], F32)  # packed reciprocals

            xT = xpool.tile([128, 8, N], F32R)

            xT_r = xT_d.rearrange("(o p) t -> p o t", p=128)
            nc.sync.dma_start(xT[:, 0, :], xT_r[:, 0, :])
            wqk_r = wqk_d.rearrange("(o p) d -> p o d", p=128)
            for o in range(8):
                nc.sync.dma_start(wqk[:, o, :], wqk_r[:, o, :])
            for o in range(1, 8):
                nc.sync.dma_start(xT[:, o, :], xT_r[:, o, :])
            nc.sync.dma_start(wv[:], wv_d.rearrange("(o p) d -> p o d", p=128))
            nc.sync.dma_start(bqk[:], bqk_d[:])
            nc.sync.dma_start(bvB[:], bv_d[0:1, :].to_broadcast((128, 256)))
            nc.sync.dma_start(wout[:], wout_d.rearrange("(o p) e -> p o e", p=128))
            nc.sync.dma_start(bout[:], bout_d[:])
            nc.sync.dma_start(ones[:], ones_d[:])

            # ---- phase 1: qkv projection (f32r) ----
            with tc.tile_pool(name="qkvps", bufs=2, space="PSUM") as qkvps:
                for g in range(4):  # q pair0, q pair1, k pair0, k pair1
                    for ic in range(4):
                        isl = slice(ic * IC, (ic + 1) * IC)
                        psa = qkvps.tile([128, 256], F32, tag="qka", name="psa")
                        psb = qkvps.tile([128, 256], F32, tag="qkb", name="psb")
                        for o in range(8):
                            nc.tensor.matmul(
                                psa[:],
                                wqk[:, o, g * 128 : (g + 1) * 128],
                                xT[:, o, ic * IC : ic * IC + 256],
                                start=(o == 0),
                                stop=(o == 7),
                            )
                            nc.tensor.matmul(
                                psb[:],
                                wqk[:, o, g * 128 : (g + 1) * 128],
                                xT[:, o, ic * IC + 256 : ic * IC + 512],
                                start=(o == 0),
                                stop=(o == 7),
                            )
                        nc.vector.tensor_scalar_add(
                            qkT[:, g, ic * IC : ic * IC + 256],
                            psa[:],
                            bqk[:, g : g + 1],
                        )
                        nc.vector.tensor_scalar_add(
                            qkT[:, g, ic * IC + 256 : ic * IC + 512],
                            psb[:],
                            bqk[:, g : g + 1],
                        )
                for tt in range(JT):  # v tok-major
                    ps = qkvps.tile([128, 256], F32, tag="v")
                    for o in range(8):
                        nc.tensor.matmul(
                            ps[:],
                            xT[:, o, tt * 128 : (tt + 1) * 128],
                            wv[:, o, :],
                            start=(o == 0),
                            stop=(o == 7),
                        )
                    nc.vector.tensor_tensor(
                        Vsb[:, tt, :], ps[:], bvB[:], mybir.AluOpType.add
                    )

            # ---- phases 2+3: attention with pipelined normalize/out-proj ----
            with (
                tc.tile_pool(name="spool", bufs=2, space="PSUM") as spool,
                tc.tile_pool(name="zpool", bufs=1, space="PSUM") as zpool,
                tc.tile_pool(name="dpool", bufs=2, space="PSUM") as dpool,
                tc.tile_pool(name="opsum", bufs=1, space="PSUM") as opsum,
                tc.tile_pool(name="ppool", bufs=4) as ppool,
                tc.tile_pool(name="rpool", bufs=2) as rpool,
                tc.tile_pool(name="ypool", bufs=3) as ypool,
            ):
                rounds = [
                    (ic, p, jg)
                    for ic in range(4)
                    for jg in range(JT // JG)
                    for p in range(2)
                ]
                zs = {}
                dens = {}
                pend = None  # (ic, p, jg, p_lo, p_hi)

                def emit_avden(ic, p, jg, p_lo, p_hi):
                    isl = slice(ic * IC, (ic + 1) * IC)
                    if (ic, p) not in zs:
                        zs[(ic, p)] = zpool.tile(
                            [128, IC], F32, tag=f"z{p}", name=f"z{p}"
                        )
                    if ic not in dens:
                        dens[ic] = zpool.tile(
                            [128, IC], F32, tag="den", name="den"
                        )
                    z = zs[(ic, p)]
                    den = dens[ic]
                    hh_lo, hh_hi = 2 * p, 2 * p + 1
                    for g in range(JG):
                        j = jg * JG + g
                        first = j == 0
                        last = j == JT - 1
                        nc.tensor.matmul(
                            z[0:64, :],
                            Vsb[:, j, hh_lo * 64 : hh_lo * 64 + 64],
                            p_lo[:, g, :],
                            start=first,
                            stop=last,
                        )
                        nc.tensor.matmul(
                            z[64:128, :],
                            Vsb[:, j, hh_hi * 64 : hh_hi * 64 + 64],
                            p_hi[:, g, :],
                            start=first,
                            stop=last,
                        )
                        nc.tensor.matmul(
                            den[32 * hh_lo : 32 * hh_lo + 1, :],
                            ones[:],
                            p_lo[:, g, :],
                            start=first,
                            stop=last,
                            tile_position=(0, 32 * hh_lo),
                        )
                        nc.tensor.matmul(
                            den[32 * hh_hi : 32 * hh_hi + 1, :],
                            ones[:],
                            p_hi[:, g, :],
                            start=first,
                            stop=last,
                            tile_position=(0, 32 * hh_hi),
                        )
                    if jg == JT // JG - 1 and p == 1:
                        # i-chunk complete: stage Z to SBUF, den rows out
                        for pp in range(2):
                            nc.vector.tensor_copy(
                                Zsb[:, pp, isl], zs[(ic, pp)][:]
                            )
                        nc.vector.tensor_copy(densb[:], den[0:97, :])
                        for hh in range(4):
                            nc.sync.dma_start(
                                scr_d[hh : hh + 1, isl],
                                densb[32 * hh : 32 * hh + 1, :],
                            )

                def emit_recip_normalize(ic):
                    # repack this chunk's 4x512 denominators as [.., 64]
                    # (head hh -> rows 32hh..32hh+8), one cheap reciprocal,
                    # bounce back to DRAM, broadcast, normalize.
                    isl = slice(ic * IC, (ic + 1) * IC)
                    rp = rpool.tile([104, DH], F32, tag="rp", name="rp")
                    rq = rpool.tile([104, DH], F32, tag="rq", name="rq")
                    for hh in range(4):
                        nc.sync.dma_start(
                            rp[32 * hh : 32 * hh + 8, :],
                            scr_d[hh, isl].rearrange("(a b) -> a b", b=DH),
                        )
                    nc.vector.reciprocal(rq[:], rp[:])
                    for hh in range(4):
                        nc.sync.dma_start(
                            scr2_d[hh, isl].rearrange("(a b) -> a b", b=DH),
                            rq[32 * hh : 32 * hh + 8, :],
                        )
                    for p in range(2):
                        rb = rpool.tile([128, IC], F32, tag="rb", name="rb")
                        nc.sync.dma_start(
                            rb[0:64, :],
                            scr2_d[2 * p : 2 * p + 1, isl].to_broadcast((64, IC)),
                        )
                        nc.sync.dma_start(
                            rb[64:128, :],
                            scr2_d[2 * p + 1 : 2 * p + 2, isl].to_broadcast(
                                (64, IC)
                            ),
                        )
                        nc.vector.tensor_tensor(
                            O2T[:, p, isl],
                            Zsb[:, p, isl],
                            rb[:],
                            mybir.AluOpType.mult,
                        )

                def emit_outproj_et(ic, et):
                    isl = slice(ic * IC, (ic + 1) * IC)
                    esl = slice(et * 128, (et + 1) * 128)
                    yps = opsum.tile([128, IC], F32, name="yps")
                    nc.tensor.matmul(
                        yps[:],
                        wout[:, 0, esl],
                        O2T[:, 0, isl],
                        start=True,
                        stop=False,
                    )
                    nc.tensor.matmul(
                        yps[:],
                        wout[:, 1, esl],
                        O2T[:, 1, isl],
                        start=False,
                        stop=True,
                    )
                    yt = ypool.tile([128, IC], F32, name="yt")
                    nc.vector.tensor_scalar_add(
                        yt[:], yps[:], bout[:, et : et + 1]
                    )
                    nc.sync.dma_start(yT_d[esl, isl], yt[:])

                for ic, p, jg in rounds:
                    isl = slice(ic * IC, (ic + 1) * IC)
                    # S matmuls (bf16, row-group packed pairs)
                    s_lo = spool.tile([128, JG, IC], F32, tag="S", name="s_lo")
                    s_hi = spool.tile([128, JG, IC], F32, tag="S", name="s_hi")
                    for g in range(JG):
                        j = jg * JG + g
                        jsl = slice(j * 128, (j + 1) * 128)
                        nc.tensor.matmul(
                            s_lo[:, g, :],
                            qkT[0:64, 2 + p, jsl],
                            qkT[0:64, p, isl],
                            start=True,
                            stop=True,
                        )
                        nc.tensor.matmul(
                            s_hi[:, g, :],
                            qkT[64:128, 2 + p, jsl],
                            qkT[64:128, p, isl],
                            start=True,
                            stop=True,
                        )
                    # previous round's AV/den (overlaps this round's exp);
                    # when it closes an i-chunk, queue that chunk's recip
                    if pend is not None:
                        emit_avden(*pend)
                        if pend[2] == JT // JG - 1 and pend[1] == 1:
                            emit_recip_normalize(pend[0])
                    p_lo = ppool.tile([128, JG, IC], BF16, tag="P", name="p_lo")
                    p_hi = ppool.tile([128, JG, IC], BF16, tag="P", name="p_hi")
                    nc.scalar.activation(
                        p_lo[:], s_lo[:], mybir.ActivationFunctionType.Exp,
                        scale=SCALE,
                    )
                    nc.scalar.activation(
                        p_hi[:], s_hi[:], mybir.ActivationFunctionType.Exp,
                        scale=SCALE,
                    )
                    pend = (ic, p, jg, p_lo, p_hi)
                emit_avden(*pend)
                emit_recip_normalize(3)
                for ic in range(4):
                    for et in range(8):
                        emit_outproj_et(ic, et)

    nc.finalize()
    return nc


def _get_nc():
    global _NC
    if _NC is None:
        _NC = _build()
    return _NC


def kernel(x, W_qkv, b_qkv, W_out, b_out):
    x = np.asarray(x, dtype=np.float32)
    W_qkv = np.asarray(W_qkv, dtype=np.float32)
    b_qkv = np.asarray(b_qkv, dtype=np.float32)
    W_out = np.asarray(W_out, dtype=np.float32)
    b_out = np.asarray(b_out, dtype=np.float32)
    B = x.shape[0]
    assert x.shape == (B, N, E) and B == 2

    ones = np.ones((128, 1), ml_dtypes.bfloat16)
    xT_by_batch = [np.ascontiguousarray(x[b].T) for b in range(B)]

    in_maps = []
    for c in range(NCORES):
        b, hg = divmod(c, 4)
        h0 = 4 * hg
        qc = slice(64 * h0, 64 * h0 + 256)
        kc = slice(E + 64 * h0, E + 64 * h0 + 256)
        vc = slice(2 * E + 64 * h0, 2 * E + 64 * h0 + 256)
        Wqk = np.ascontiguousarray(
            np.concatenate([W_qkv[:, qc], W_qkv[:, kc]], axis=1)
        )  # [E, 512] = [q pair0 | q pair1 | k pair0 | k pair1]
        Wv = np.ascontiguousarray(W_qkv[:, vc])  # [E, 256]
        bqk = np.ascontiguousarray(
            np.stack(
                [
                    b_qkv[64 * h0 : 64 * h0 + 128],
                    b_qkv[64 * h0 + 128 : 64 * h0 + 256],
                    b_qkv[E + 64 * h0 : E + 64 * h0 + 128],
                    b_qkv[E + 64 * h0 + 128 : E + 64 * h0 + 256],
                ],
                axis=1,
            )
        )  # [128, 4]
        bv = np.ascontiguousarray(b_qkv[vc][None, :])  # [1, 256]
        Wout = np.ascontiguousarray(W_out[64 * h0 : 64 * h0 + 256, :])  # [256, E]
        if hg == 0:
            bout = np.ascontiguousarray(b_out.reshape(8, 128).T)
        else:
            bout = np.zeros((128, 8), np.float32)
        in_maps.append(
            {
                "xT": xT_by_batch[b],
                "Wqk": Wqk,
                "Wv": Wv,
                "bqk": bqk,
                "bv": bv,
                "Wout": Wout,
                "bout": bout,
                "ones": ones,
            }
        )

    global _last_in_maps
    _last_in_maps = in_maps
    res = run_bass_kernel_spmd(_get_nc(), in_maps, core_ids=list(range(NCORES)))

    out = np.zeros((B, N, E), np.float32)
    for c in range(NCORES):
        b = c // 4
        out[b] += res.results[c]["yT"].T
    return out


# revision 16
# speedup vs baseline: 1.0997x; 1.0997x over previous
"""Multi-head attention (B=2, N=2048, E=1024, H=16) on 8 trn2 NeuronCores.

Sharding: data-parallel over batch (2 groups of 4 cores) x tensor-parallel
over heads (4 heads per core). Each core computes, for its batch b and its
4 heads: qkv projection (its W_qkv column slices), head-parallel attention,
and a partial output projection (its W_out row slice), returning a partial
y^T [1024, 2048]. The host sums the 4 partials per batch and transposes.

Device layout (no transposes anywhere on device):
  - x is passed pre-transposed as xT [E, N] (host transpose).
  - q, k are produced d-major in bf16: qkT [128, 4, N]: partition = d of a
    head PAIR (rows 0:64 head-lo, 64:128 head-hi); groups = [q pair0,
    q pair1, k pair0, k pair1]. The qkv projection itself runs in f32r
    (full f32 inputs, relaxed single-pass matmul) so only the final
    rounding of q/k/v is bf16.
  - v is produced tok-major: V [128(tok), 16(tok-tile), 256(head d)] bf16.
  - scores are computed transposed S^T[j, i] = k_j . q_i via bf16 matmuls,
    two heads concurrently via K=64 row groups.
  - softmax: exp on ScalarE (scale folded in, no max subtraction -- scores
    are O(1) by construction); denominator via ones-column matmuls (M=1,
    col-groups 0/32/64/96); P^T is bf16.
  - attention out Z is staged unnormalized to SBUF; all reciprocals are
    batched at the end on a [128, 64] repack (DVE reciprocal is ~8
    cycles/elem/lane, so use all 128 lanes), then broadcast via a DRAM
    bounce and applied as a multiply.
  - out-projection computes y^T [e, i] (f32r) with W_out stationary, bias
    added per-partition during the PSUM->SBUF copy.

Emission is software-pipelined: within the attention loop, round r's
S-matmuls are emitted before round r-1's AV/den matmuls so the PE never
head-of-line blocks on the exp that feeds AV.
"""

import numpy as np
import ml_dtypes

import concourse.mybir as mybir
import concourse.tile as tile
from concourse import bacc
from concourse.bass_utils import run_bass_kernel_spmd

F32R = mybir.dt.float32r
F32 = mybir.dt.float32
BF16 = mybir.dt.bfloat16

E = 1024  # embed dim
N = 2048  # sequence length
DH = 64  # head dim
SCALE = DH**-0.5
IC = 512  # i-chunk (queries per block)
JT = N // 128  # 16 j-tiles (keys)
JG = 2  # j-tiles per exp group (ACT free dim = JG*IC = 1024)
NCORES = 8

_NC = None  # cached compiled program (same for every core / call)


def _build():
    nc = bacc.Bacc("TRN2", target_bir_lowering=False, debug=False)
    xT_d = nc.dram_tensor("xT", [E, N], F32R, kind="ExternalInput")
    wqk_d = nc.dram_tensor("Wqk", [E, 512], F32R, kind="ExternalInput")
    wv_d = nc.dram_tensor("Wv", [E, 256], F32R, kind="ExternalInput")
    bqk_d = nc.dram_tensor("bqk", [128, 4], F32, kind="ExternalInput")
    bv_d = nc.dram_tensor("bv", [1, 256], F32, kind="ExternalInput")
    wout_d = nc.dram_tensor("Wout", [256, E], F32R, kind="ExternalInput")
    bout_d = nc.dram_tensor("bout", [128, 8], F32, kind="ExternalInput")
    ones_d = nc.dram_tensor("ones", [128, 1], BF16, kind="ExternalInput")
    yT_d = nc.dram_tensor("yT", [E, N], F32, kind="ExternalOutput")
    scr_d = nc.dram_tensor("den_scratch", [4, N], F32)  # internal bounce
    scr2_d = nc.dram_tensor("recip_scratch", [4, N], F32)  # internal bounce

    with tile.TileContext(nc) as tc:
        with (
            tc.tile_pool(name="persist", bufs=1) as persist,
            tc.tile_pool(name="xpool", bufs=1) as xpool,
        ):
            # ---- persistent SBUF tensors ----
            wqk = persist.tile([128, 8, 512], F32R)
            wv = persist.tile([128, 8, 256], F32R)
            bqk = persist.tile([128, 4], F32)
            bvB = persist.tile([128, 256], F32)
            wout = persist.tile([128, 2, E], F32R)
            bout = persist.tile([128, 8], F32)
            ones = persist.tile([128, 1], BF16)
            qkT = persist.tile([128, 4, N], BF16)  # q/k d-major
            Vsb = persist.tile([128, JT, 256], BF16)  # v tok-major
            Zsb = persist.tile([128, 2, N], F32R)  # unnormalized attn out
            O2T = persist.tile([128, 2, N], F32R)  # normalized attn out
            rp = persist.tile([128, DH], F32)  # packed denominators
            densb = persist.tile([97, IC], F32)  # den staging (rows 0/32/64/96)
            rq = persist.tile([128, DH], F32)  # packed reciprocals

            xT = xpool.tile([128, 8, N], F32R)

            xT_r = xT_d.rearrange("(o p) t -> p o t", p=128)
            nc.sync.dma_start(xT[:, 0, :], xT_r[:, 0, :])
            wqk_r = wqk_d.rearrange("(o p) d -> p o d", p=128)
            for o in range(8):
                nc.sync.dma_start(wqk[:, o, :], wqk_r[:, o, :])
            for o in range(1, 8):
                nc.sync.dma_start(xT[:, o, :], xT_r[:, o, :])
            nc.sync.dma_start(wv[:], wv_d.rearrange("(o p) d -> p o d", p=128))
            nc.sync.dma_start(bqk[:], bqk_d[:])
            nc.sync.dma_start(bvB[:], bv_d[0:1, :].to_broadcast((128, 256)))
            nc.sync.dma_start(wout[:], wout_d.rearrange("(o p) e -> p o e", p=128))
            nc.sync.dma_start(bout[:], bout_d[:])
            nc.sync.dma_start(ones[:], ones_d[:])

            # ---- phase 1: qkv projection (f32r) ----
            with tc.tile_pool(name="qkvps", bufs=2, space="PSUM") as qkvps:
                for g in range(4):  # q pair0, q pair1, k pair0, k pair1
                    for ic in range(4):
                        isl = slice(ic * IC, (ic + 1) * IC)
                        psa = qkvps.tile([128, 256], F32, tag="qka", name="psa")
                        psb = qkvps.tile([128, 256], F32, tag="qkb", name="psb")
                        for o in range(8):
                            nc.tensor.matmul(
                                psa[:],
                                wqk[:, o, g * 128 : (g + 1) * 128],
                                xT[:, o, ic * IC : ic * IC + 256],
                                start=(o == 0),
                                stop=(o == 7),
                            )
                            nc.tensor.matmul(
                                psb[:],
                                wqk[:, o, g * 128 : (g + 1) * 128],
                                xT[:, o, ic * IC + 256 : ic * IC + 512],
                                start=(o == 0),
                                stop=(o == 7),
                            )
                        nc.vector.tensor_scalar_add(
                            qkT[:, g, ic * IC : ic * IC + 256],
                            psa[:],
                            bqk[:, g : g + 1],
                        )
                        nc.vector.tensor_scalar_add(
                            qkT[:, g, ic * IC + 256 : ic * IC + 512],
                            psb[:],
                            bqk[:, g : g + 1],
                        )
                for tt in range(JT):  # v tok-major
                    ps = qkvps.tile([128, 256], F32, tag="v")
                    for o in range(8):
                        nc.tensor.matmul(
                            ps[:],
                            xT[:, o, tt * 128 : (tt + 1) * 128],
                            wv[:, o, :],
                            start=(o == 0),
                            stop=(o == 7),
                        )
                    nc.vector.tensor_tensor(
                        Vsb[:, tt, :], ps[:], bvB[:], mybir.AluOpType.add
                    )

            # ---- phases 2+3: attention with pipelined normalize/out-proj ----
            with (
                tc.tile_pool(name="spool", bufs=2, space="PSUM") as spool,
                tc.tile_pool(name="zpool", bufs=1, space="PSUM") as zpool,
                tc.tile_pool(name="dpool", bufs=2, space="PSUM") as dpool,
                tc.tile_pool(name="ppool", bufs=4) as ppool,
                tc.tile_pool(name="rpool", bufs=2) as rpool,
                tc.tile_pool(name="ypool", bufs=3) as ypool,
            ):
                rounds = [
                    (ic, p, jg)
                    for ic in range(4)
                    for jg in range(JT // JG)
                    for p in range(2)
                ]
                zs = {}
                dens = {}
                pend = None  # (ic, p, jg, p_lo, p_hi)

                def emit_avden(ic, p, jg, p_lo, p_hi):
                    isl = slice(ic * IC, (ic + 1) * IC)
                    if (ic, p) not in zs:
                        zs[(ic, p)] = zpool.tile(
                            [128, IC], F32, tag=f"z{p}", name=f"z{p}"
                        )
                    if ic not in dens:
                        dens[ic] = zpool.tile(
                            [128, IC], F32, tag="den", name="den"
                        )
                    z = zs[(ic, p)]
                    den = dens[ic]
                    hh_lo, hh_hi = 2 * p, 2 * p + 1
                    for g in range(JG):
                        j = jg * JG + g
                        first = j == 0
                        last = j == JT - 1
                        nc.tensor.matmul(
                            z[0:64, :],
                            Vsb[:, j, hh_lo * 64 : hh_lo * 64 + 64],
                            p_lo[:, g, :],
                            start=first,
                            stop=last,
                        )
                        nc.tensor.matmul(
                            z[64:128, :],
                            Vsb[:, j, hh_hi * 64 : hh_hi * 64 + 64],
                            p_hi[:, g, :],
                            start=first,
                            stop=last,
                        )
                        nc.tensor.matmul(
                            den[32 * hh_lo : 32 * hh_lo + 1, :],
                            ones[:],
                            p_lo[:, g, :],
                            start=first,
                            stop=last,
                            tile_position=(0, 32 * hh_lo),
                        )
                        nc.tensor.matmul(
                            den[32 * hh_hi : 32 * hh_hi + 1, :],
                            ones[:],
                            p_hi[:, g, :],
                            start=first,
                            stop=last,
                            tile_position=(0, 32 * hh_hi),
                        )
                    if jg == JT // JG - 1 and p == 1:
                        # i-chunk complete: stage Z to SBUF, den rows out
                        for pp in range(2):
                            nc.vector.tensor_copy(
                                Zsb[:, pp, isl], zs[(ic, pp)][:]
                            )
                        nc.vector.tensor_copy(densb[:], den[0:97, :])
                        for hh in range(4):
                            nc.sync.dma_start(
                                scr_d[hh : hh + 1, isl],
                                densb[32 * hh : 32 * hh + 1, :],
                            )

                def emit_recip_normalize(ic):
                    # repack this chunk's 4x512 denominators as [.., 64]
                    # (head hh -> rows 32hh..32hh+8), one cheap reciprocal,
                    # bounce back to DRAM, broadcast, normalize.
                    isl = slice(ic * IC, (ic + 1) * IC)
                    rp = rpool.tile([104, DH], F32, tag="rp", name="rp")
                    rq = rpool.tile([104, DH], F32, tag="rq", name="rq")
                    for hh in range(4):
                        nc.sync.dma_start(
                            rp[32 * hh : 32 * hh + 8, :],
                            scr_d[hh, isl].rearrange("(a b) -> a b", b=DH),
                        )
                    nc.vector.reciprocal(rq[:], rp[:])
                    for hh in range(4):
                        nc.sync.dma_start(
                            scr2_d[hh, isl].rearrange("(a b) -> a b", b=DH),
                            rq[32 * hh : 32 * hh + 8, :],
                        )
                    for p in range(2):
                        rb = rpool.tile([128, IC], F32, tag="rb", name="rb")
                        nc.sync.dma_start(
                            rb[0:64, :],
                            scr2_d[2 * p : 2 * p + 1, isl].to_broadcast((64, IC)),
                        )
                        nc.sync.dma_start(
                            rb[64:128, :],
                            scr2_d[2 * p + 1 : 2 * p + 2, isl].to_broadcast(
                                (64, IC)
                            ),
                        )
                        nc.vector.tensor_tensor(
                            O2T[:, p, isl],
                            Zsb[:, p, isl],
                            rb[:],
                            mybir.AluOpType.mult,
                        )

                def emit_outproj_et(ic, et, opsum, ypool):
                    isl = slice(ic * IC, (ic + 1) * IC)
                    esl = slice(et * 128, (et + 1) * 128)
                    yps = opsum.tile([128, IC], F32, name="yps")
                    nc.tensor.matmul(
                        yps[:],
                        wout[:, 0, esl],
                        O2T[:, 0, isl],
                        start=True,
                        stop=False,
                    )
                    nc.tensor.matmul(
                        yps[:],
                        wout[:, 1, esl],
                        O2T[:, 1, isl],
                        start=False,
                        stop=True,
                    )
                    yt = ypool.tile([128, IC], F32, name="yt")
                    nc.vector.tensor_scalar_add(
                        yt[:], yps[:], bout[:, et : et + 1]
                    )
                    nc.sync.dma_start(yT_d[esl, isl], yt[:])

                for ic, p, jg in rounds:
                    isl = slice(ic * IC, (ic + 1) * IC)
                    # S matmuls (bf16, row-group packed pairs)
                    s_lo = spool.tile([128, JG, IC], F32, tag="S", name="s_lo")
                    s_hi = spool.tile([128, JG, IC], F32, tag="S", name="s_hi")
                    for g in range(JG):
                        j = jg * JG + g
                        jsl = slice(j * 128, (j + 1) * 128)
                        nc.tensor.matmul(
                            s_lo[:, g, :],
                            qkT[0:64, 2 + p, jsl],
                            qkT[0:64, p, isl],
                            start=True,
                            stop=True,
                        )
                        nc.tensor.matmul(
                            s_hi[:, g, :],
                            qkT[64:128, 2 + p, jsl],
                            qkT[64:128, p, isl],
                            start=True,
                            stop=True,
                        )
                    # previous round's AV/den (overlaps this round's exp);
                    # when it closes an i-chunk, queue that chunk's recip
                    if pend is not None:
                        emit_avden(*pend)
                        if pend[2] == JT // JG - 1 and pend[1] == 1:
                            emit_recip_normalize(pend[0])
                    p_lo = ppool.tile([128, JG, IC], BF16, tag="P", name="p_lo")
                    p_hi = ppool.tile([128, JG, IC], BF16, tag="P", name="p_hi")
                    nc.scalar.activation(
                        p_lo[:], s_lo[:], mybir.ActivationFunctionType.Exp,
                        scale=SCALE,
                    )
                    nc.scalar.activation(
                        p_hi[:], s_hi[:], mybir.ActivationFunctionType.Exp,
                        scale=SCALE,
                    )
                    pend = (ic, p, jg, p_lo, p_hi)
                emit_avden(*pend)
                emit_recip_normalize(3)
            # ---- phase 3: output projection (attention banks free) ----
            with (
                tc.tile_pool(name="opsum", bufs=4, space="PSUM") as opsum,
                tc.tile_pool(name="ypool2", bufs=4) as ypool2,
            ):
                for ic in range(4):
                    for et in range(8):
                        emit_outproj_et(ic, et, opsum, ypool2)

    nc.finalize()
    return nc


def _get_nc():
    global _NC
    if _NC is None:
        _NC = _build()
    return _NC


def kernel(x, W_qkv, b_qkv, W_out, b_out):
    x = np.asarray(x, dtype=np.float32)
    W_qkv = np.asarray(W_qkv, dtype=np.float32)
    b_qkv = np.asarray(b_qkv, dtype=np.float32)
    W_out = np.asarray(W_out, dtype=np.float32)
    b_out = np.asarray(b_out, dtype=np.float32)
    B = x.shape[0]
    assert x.shape == (B, N, E) and B == 2

    ones = np.ones((128, 1), ml_dtypes.bfloat16)
    xT_by_batch = [np.ascontiguousarray(x[b].T) for b in range(B)]

    in_maps = []
    for c in range(NCORES):
        b, hg = divmod(c, 4)
        h0 = 4 * hg
        qc = slice(64 * h0, 64 * h0 + 256)
        kc = slice(E + 64 * h0, E + 64 * h0 + 256)
        vc = slice(2 * E + 64 * h0, 2 * E + 64 * h0 + 256)
        Wqk = np.ascontiguousarray(
            np.concatenate([W_qkv[:, qc], W_qkv[:, kc]], axis=1)
        )  # [E, 512] = [q pair0 | q pair1 | k pair0 | k pair1]
        Wv = np.ascontiguousarray(W_qkv[:, vc])  # [E, 256]
        bqk = np.ascontiguousarray(
            np.stack(
                [
                    b_qkv[64 * h0 : 64 * h0 + 128],
                    b_qkv[64 * h0 + 128 : 64 * h0 + 256],
                    b_qkv[E + 64 * h0 : E + 64 * h0 + 128],
                    b_qkv[E + 64 * h0 + 128 : E + 64 * h0 + 256],
                ],
                axis=1,
            )
        )  # [128, 4]
        bv = np.ascontiguousarray(b_qkv[vc][None, :])  # [1, 256]
        Wout = np.ascontiguousarray(W_out[64 * h0 : 64 * h0 + 256, :])  # [256, E]
        if hg == 0:
            bout = np.ascontiguousarray(b_out.reshape(8, 128).T)
        else:
            bout = np.zeros((128, 8), np.float32)
        in_maps.append(
            {
                "xT": xT_by_batch[b],
                "Wqk": Wqk,
                "Wv": Wv,
                "bqk": bqk,
                "bv": bv,
                "Wout": Wout,
                "bout": bout,
                "ones": ones,
            }
        )

    global _last_in_maps
    _last_in_maps = in_maps
    res = run_bass_kernel_spmd(_get_nc(), in_maps, core_ids=list(range(NCORES)))

    out = np.zeros((B, N, E), np.float32)
    for c in range(NCORES):
        b = c // 4
        out[b] += res.results[c]["yT"].T
    return out
